# revision 14
# baseline (speedup 1.0000x reference)
"""2-layer multi-head GAT on 8 TRN2 NeuronCores (Bass/Tile), v4b.

Sharding: destination-node blocks. Core i owns nodes [i*NPC, (i+1)*NPC) and
all edges whose dst lands there, so edge softmax + aggregation are core-local.

v4 vs v3:
- No t1 AllGather: h is replicated (host side, free) and every core computes
  z1 for ALL nodes into a private DRAM table (dense bf16 matmuls are cheap;
  the v3 t1 AllGather was a ~230us serial stall).
- All-bf16 dense path with DMA-transpose loads of h (no fp32 matmuls).
- Per-core degree-balanced dst->window assignment (snake over sorted degree);
  output rows are unpermuted on the host. gsrc indexes the PERMUTED rows.
- One shared edge structure for both layers, piece-major node numbering
  everywhere.
- Both one-hot orientations (forward for aggregation, transposed for the
  s_dst expansion) stream from DRAM as fp8 — no on-chip one-hot build.
- Per-window edge phase is software-pipelined 2 deep.
- t2 AllGather moves compact 66-col rows; each piece is expanded to the
  256B-stride gather table by a local DRAM->DRAM DMA.
"""
import sys
sys.path.insert(0, "/opt/trn_rl_repo")

import numpy as np
import ml_dtypes

import concourse.bass as bass
import concourse.bacc as bacc
import concourse.tile as tile
import concourse.mybir as mybir
from concourse.bass_utils import run_bass_kernel_spmd
from concourse.masks import make_identity

F32 = mybir.dt.float32
BF16 = mybir.dt.bfloat16
F8 = mybir.dt.float8e4
I16 = mybir.dt.int16

NCORES = 8
HALF = 32768
BFNP = ml_dtypes.bfloat16
F8NP = ml_dtypes.float8_e4m3

OT_DT = F8          # dtype of the streamed one-hots
OT_NP = F8NP

PIPE = 2            # software pipeline depth of the edge phases


def _round_up(x, m):
    return (x + m - 1) // m * m


# ----------------------------------------------------------------- host prep

def preprocess(h, src, dst, W1, a1, W2, a2):
    N, IN_DIM = h.shape
    HEADS, _, HID = W1.shape
    OUT = W2.shape[1]
    ZC = HEADS * HID
    npc = N // NCORES
    rows = _round_up(npc + 1, 128)
    NW = rows // 128
    GROWS = NCORES * rows  # 50176

    # ---- weight folding
    w1cat = np.transpose(W1, (1, 0, 2)).reshape(IN_DIM, ZC)
    w1s = np.stack([W1[hh] @ a1[hh, :HID] for hh in range(HEADS)], 1)
    w1d = np.stack([W1[hh] @ a1[hh, HID:] for hh in range(HEADS)], 1)
    wc1 = np.concatenate([w1cat, w1s], 1).astype(BFNP)          # [128, 260]
    wc2 = np.concatenate([W2, (W2 @ a2[:OUT])[:, None],
                          (W2 @ a2[OUT:])[:, None]], 1).astype(BFNP)  # [256,66]
    # crow must match the bf16 weights actually used on device
    crow = np.tile(wc2.astype(np.float32).sum(0)[None, :],
                   (128, 1)).astype(np.float32)

    # ---- piece-major global numbering (used for t1_priv AND t2 AllGather)
    ptiles = [NW - 7 * (NW // 8)] + [NW // 8] * 7
    S = np.cumsum([0] + ptiles)[:-1] * 128
    SZ = np.array(ptiles) * 128
    pieces = [(int(S[p]), int(SZ[p])) for p in range(8)]

    # ---- per-core degree-balanced dst -> (window, slot) assignment
    core_of = dst // npc
    deg = np.zeros((NCORES, npc), dtype=np.int64)
    for c in range(NCORES):
        np.add.at(deg[c], (dst[core_of == c] - c * npc).astype(np.int64), 1)

    # snake assignment of degree-sorted dsts -> near-uniform window loads
    win_of = np.zeros((NCORES, npc), dtype=np.int64)
    slot_of = np.zeros((NCORES, npc), dtype=np.int64)
    node_at = np.full((NCORES, NW * 128), -1, dtype=np.int64)  # local node idx
    ii = np.arange(npc)
    jj2 = ii % (2 * NW)
    wsnake = np.where(jj2 < NW, jj2, 2 * NW - 1 - jj2)
    slotsnake = ii // (2 * NW) * 2 + (jj2 >= NW)
    for c in range(NCORES):
        order = np.argsort(-deg[c], kind="stable")
        win_of[c, order] = wsnake
        slot_of[c, order] = slotsnake
        node_at[c, wsnake * 128 + slotsnake] = order

    # ---- permuted local row of every node, and its piece-major global row.
    # t1/t2 table rows are in (window, slot) order per owner core, so gsrc
    # must be derived from the permuted position, not the raw local index.
    src64 = src.astype(np.int64)
    so = src64 // npc
    sd = src64 % npc
    pl = win_of[so, sd] * 128 + slot_of[so, sd]
    pp = np.searchsorted(S, pl, side="right") - 1
    gsrc_all = 8 * S[pp] + so * SZ[pp] + (pl - S[pp])

    # ---- edge partition per core, chunk structure shared by both layers
    pc = []
    for c in range(NCORES):
        m = core_of == c
        dl = (dst[m] - c * npc).astype(np.int64)
        pc.append((win_of[c][dl], slot_of[c][dl], gsrc_all[m]))

    low_cnt = np.zeros((NCORES, NW), dtype=np.int64)
    high_cnt = np.zeros((NCORES, NW), dtype=np.int64)
    for c in range(NCORES):
        w, _, gs = pc[c]
        hi = gs >= HALF
        np.add.at(low_cnt[c], w[~hi], 1)
        np.add.at(high_cnt[c], w[hi], 1)
    KL = np.maximum(1, np.ceil(low_cnt.max(0) / 128.0).astype(np.int64))
    KH = np.ceil(high_cnt.max(0) / 128.0).astype(np.int64)
    chunks = KL + KH
    bases = (np.concatenate([[0], np.cumsum(chunks)]) * 128).astype(np.int64)
    total_pos = int(bases[-1])

    eidx = np.zeros((NCORES, total_pos), dtype=np.int16)
    wloc_flat = np.full((NCORES, total_pos), -1.0, dtype=np.float32)
    for c in range(NCORES):
        w, sl, gs = pc[c]
        hi = (gs >= HALF).astype(np.int64)
        key = w * 2 + hi
        order = np.argsort(key, kind="stable")
        ks = key[order]
        new = np.ones(len(ks), dtype=bool)
        new[1:] = ks[1:] != ks[:-1]
        starts = np.flatnonzero(new)
        lens = np.diff(np.append(starts, len(ks)))
        within = np.arange(len(ks)) - np.repeat(starts, lens)
        w_o, h_o, sl_o = w[order], hi[order], sl[order]
        pos = bases[w_o] + h_o * KL[w_o] * 128 + within
        eidx[c, pos] = (gs[order] - h_o * HALF).astype(np.int16)
        wloc_flat[c, pos] = sl_o.astype(np.float32)

    jj = np.arange(128, dtype=np.float32)
    # transposed one-hot: otab[j, pos] = (wloc[pos] == j)
    otab = (wloc_flat[:, None, :] == jj[None, :, None]).astype(OT_NP)
    # forward one-hot: ofab[p, ck*128+j] = (wloc[ck*128+p] == j)
    wl3 = wloc_flat.reshape(NCORES, total_pos // 128, 128)  # [c, ck, p]
    ofab = (wl3.transpose(0, 2, 1)[:, :, :, None]
            == jj[None, None, None, :]).astype(OT_NP)       # [c, p, ck, j]
    ofab = ofab.reshape(NCORES, 128, total_pos)

    windows = [(int(bases[w]), int(KL[w]), int(KH[w])) for w in range(NW)]
    struct = dict(
        N=N, E=src.shape[0], IN_DIM=IN_DIM, HEADS=HEADS, HID=HID, OUT=OUT,
        npc=npc, rows=rows, grows=GROWS, total_pos=total_pos,
        windows=windows, pieces=pieces,
    )

    hb = h.astype(BFNP)
    # piece-major replicated h (identical for all cores), rows in each
    # owner's permuted (window, slot) order to match t1/t2 row numbering
    h_pm = np.zeros((GROWS, IN_DIM), dtype=BFNP)
    for o in range(NCORES):
        na = node_at[o]
        valid = np.flatnonzero(na >= 0)
        ppp = np.searchsorted(S, valid, side="right") - 1
        gp = 8 * S[ppp] + o * SZ[ppp] + (valid - S[ppp])
        h_pm[gp] = hb[o * npc + na[valid]]

    def idx_tile(a):
        t = a.reshape(-1, 16).T.copy()
        return np.concatenate([t] * 8, 0)

    in_maps = []
    for c in range(NCORES):
        # per-core permuted local h (window order)
        h_loc = np.zeros((rows, IN_DIM), dtype=BFNP)
        na = node_at[c]
        valid = na >= 0
        h_loc[np.flatnonzero(valid)] = hb[c * npc + na[valid]]
        in_maps.append({
            "h_pm": h_pm,
            "h_loc": h_loc,
            "eidx": idx_tile(eidx[c]),
            "otab": otab[c],
            "ofab": ofab[c],
            "wc1": wc1,
            "w1d": w1d.astype(BFNP),
            "wc2": wc2,
            "crow": crow,
        })
    return struct, in_maps, node_at


# --------------------------------------------------------------- bass graph

def build(s):
    rows, grows, total_pos = s["rows"], s["grows"], s["total_pos"]
    windows, pieces = s["windows"], s["pieces"]
    IN_DIM, HEADS, HID, OUT = s["IN_DIM"], s["HEADS"], s["HID"], s["OUT"]
    ZC = HEADS * HID
    NW = rows // 128
    GW = grows // 128          # 392 global windows
    AF = mybir.ActivationFunctionType

    nc = bacc.Bacc("TRN2", target_bir_lowering=False, debug=False,
                   num_devices=NCORES, num_swdge_queues=4)

    h_pm_in = nc.dram_tensor("h_pm", [grows, IN_DIM], BF16,
                             kind="ExternalInput")
    h_loc_in = nc.dram_tensor("h_loc", [rows, IN_DIM], BF16,
                              kind="ExternalInput")
    eidx_in = nc.dram_tensor("eidx", [128, total_pos // 16], I16,
                             kind="ExternalInput")
    otab_in = nc.dram_tensor("otab", [128, total_pos], OT_DT,
                             kind="ExternalInput")
    ofab_in = nc.dram_tensor("ofab", [128, total_pos], OT_DT,
                             kind="ExternalInput")
    wc1_in = nc.dram_tensor("wc1", [IN_DIM, ZC + HEADS], BF16,
                            kind="ExternalInput")
    w1d_in = nc.dram_tensor("w1d", [IN_DIM, HEADS], BF16,
                            kind="ExternalInput")
    wc2_in = nc.dram_tensor("wc2", [ZC, OUT + 2], BF16, kind="ExternalInput")
    crow_in = nc.dram_tensor("crow", [128, OUT + 2], F32,
                             kind="ExternalInput")
    out_ext = nc.dram_tensor("out", [rows, OUT], F32, kind="ExternalOutput")

    TROW = 384  # t1 row stride in bf16 elems (768B; 260 used)
    T2C = OUT + 2  # compact t2 row (66)

    with tile.TileContext(nc) as tc:
        with (
            tc.tile_pool(name="dram", bufs=1, space="DRAM") as dram,
            tc.tile_pool(name="const", bufs=1) as const,
        ):
            t1_priv = dram.tile([grows + 128, TROW], BF16)
            t2_loc = dram.tile([rows, T2C], BF16)
            t2_full = dram.tile([grows + 128, 128], BF16)
            t2c_full = nc.dram_tensor("t2c_full_sh", [grows, T2C],
                                      BF16, kind="Internal",
                                      addr_space="Shared").ap()

            identb = const.tile([128, 128], BF16)
            make_identity(nc, identb[:])
            wc1_t = const.tile([IN_DIM, ZC + HEADS], BF16)
            nc.sync.dma_start(wc1_t[:], wc1_in[:])
            w1d_t = const.tile([IN_DIM, HEADS], BF16)
            nc.sync.dma_start(w1d_t[:], w1d_in[:])
            wc2a = const.tile([128, OUT + 2], BF16)
            wc2b = const.tile([128, OUT + 2], BF16)
            nc.sync.dma_start(wc2a[:], wc2_in[0:128, :])
            nc.sync.dma_start(wc2b[:], wc2_in[128:256, :])
            crow = const.tile([128, OUT + 2], F32)
            nc.sync.dma_start(crow[:], crow_in[:])
            eidx_t = const.tile([128, total_pos // 16], I16)
            nc.sync.dma_start(eidx_t[:], eidx_in[:])
            s1_all = const.tile([128, NW, HEADS], BF16)
            s2_all = const.tile([128, NW], BF16)
            onesb = const.tile([128, 1], BF16)
            epsb = const.tile([128, HEADS], F32)
            nc.vector.memset(onesb[:], 1.0)
            nc.vector.memset(epsb[:], 1e-30)

            # ---------------- D1b: local dst scores --------------------
            with (
                tc.tile_pool(name="d1b", bufs=2) as d1b,
                tc.tile_pool(name="psum_sd", bufs=2, space="PSUM") as psum_sd,
            ):
                G = 4
                for g0 in range(0, NW, G):
                    gn = min(G, NW - g0)
                    hT = d1b.tile([128, gn * 128], BF16, tag="hTl")
                    nc.sync.dma_start_transpose(
                        hT[:], h_loc_in[g0 * 128:(g0 + gn) * 128, :])
                    for k in range(gn):
                        sdp = psum_sd.tile([128, HEADS], F32, tag="sd")
                        nc.tensor.matmul(sdp[:], hT[:, k * 128:(k + 1) * 128],
                                         w1d_t[:], start=True, stop=True)
                        nc.vector.tensor_copy(s1_all[:, g0 + k, :], sdp[:])

            # ---------------- D1a: z1 | s_src for ALL nodes ------------
            with (
                tc.tile_pool(name="d1a", bufs=3) as d1a,
                tc.tile_pool(name="psum_z", bufs=4, space="PSUM") as psum_z,
            ):
                G = 8
                for g0 in range(0, GW, G):
                    gn = min(G, GW - g0)
                    hT = d1a.tile([128, gn * 128], BF16, tag="hT")
                    nc.sync.dma_start_transpose(
                        hT[:], h_pm_in[g0 * 128:(g0 + gn) * 128, :])
                    t1t = d1a.tile([128, gn, ZC + HEADS], BF16, tag="t1t")
                    for k in range(gn):
                        zps = psum_z.tile([128, ZC + HEADS], F32, tag="zp")
                        nc.tensor.matmul(zps[:], hT[:, k * 128:(k + 1) * 128],
                                         wc1_t[:], start=True, stop=True)
                        if k % 2 == 0:
                            nc.vector.tensor_copy(t1t[:, k, :], zps[:])
                        else:
                            nc.scalar.copy(t1t[:, k, :], zps[:])
                    dview = t1_priv[g0 * 128:(g0 + gn) * 128, 0:ZC + HEADS]
                    nc.sync.dma_start(
                        dview.rearrange("(k p) e -> p k e", k=gn), t1t[:])

            # ------- L1 edge phase + fused D2, software-pipelined -------
            with (
                tc.tile_pool(name="l1", bufs=PIPE + 1) as l1,
                tc.tile_pool(name="l1g", bufs=PIPE + 2) as l1g,
                tc.tile_pool(name="l1o", bufs=PIPE + 2) as l1o,
                tc.tile_pool(name="l1m", bufs=2) as l1m,
                tc.tile_pool(name="d2", bufs=2) as d2,
                tc.tile_pool(name="psum_agg", bufs=2, space="PSUM") as psum_agg,
                tc.tile_pool(name="psum_qd", bufs=PIPE + 1,
                             space="PSUM") as psum_qd,
                tc.tile_pool(name="psum_z2", bufs=2, space="PSUM") as psum_z2,
                tc.tile_pool(name="psum_tp", bufs=1, space="PSUM") as psum_tp,
            ):
                qc = 0
                stage = []

                def l1_stage_a(wi):
                    nonlocal qc
                    base, KLw, KHw = windows[wi]
                    C = KLw + KHw
                    g = l1g.tile([128, C, TROW], BF16, tag="g")
                    for c0, c1, tab in ((0, KLw, 0), (KLw, C, HALF)):
                        if c1 == c0:
                            continue
                        b0, b1 = base + c0 * 128, base + c1 * 128
                        nc.gpsimd.dma_gather(
                            g[:, c0:c1, :], t1_priv[tab:, :],
                            eidx_t[:, b0 // 16:b1 // 16],
                            num_idxs=(c1 - c0) * 128,
                            num_idxs_reg=(c1 - c0) * 128,
                            elem_size=TROW, single_packet=False,
                            queue_num=qc % 4); qc += 1
                    ot = l1o.tile([128, C, 128], OT_DT, tag="ot")
                    nc.scalar.dma_start(ot[:], otab_in[:, base:base + C * 128])
                    of = l1o.tile([128, C, 128], OT_DT, tag="of")
                    nc.scalar.dma_start(of[:], ofab_in[:, base:base + C * 128])
                    qd = psum_qd.tile([128, C, HEADS], F32, tag="qd")
                    for cc in range(C):
                        nc.tensor.matmul(
                            qd[:, cc, :], ot[:, cc, :], s1_all[:, wi, :],
                            start=True, stop=True)
                    return (wi, C, g, of, qd)

                def l1_stage_b(st):
                    wi, C, g, of, qd = st
                    q = l1.tile([128, C, HEADS], F32, tag="q")
                    nc.vector.tensor_add(q[:], g[:, :, ZC:ZC + HEADS], qd[:])
                    ea = l1.tile([128, C, HEADS], BF16, tag="ea")
                    nc.scalar.activation(ea[:], q[:], AF.Exp)
                    eb = l1.tile([128, C, HEADS], BF16, tag="eb")
                    nc.scalar.activation(eb[:], q[:], AF.Exp, scale=0.01)
                    num = l1.tile([128, C, HEADS], BF16, tag="num")
                    nc.vector.tensor_max(num[:], ea[:], eb[:])

                    m = l1m.tile([128, C, ZC + HEADS], BF16, tag="m")
                    nc.vector.tensor_tensor(
                        m[:, :, 0:ZC].rearrange(
                            "p c (h x) -> p c h x", h=HEADS),
                        g[:, :, 0:ZC].rearrange(
                            "p c (h x) -> p c h x", h=HEADS),
                        num[:, :, :, None].to_broadcast((128, C, HEADS, HID)),
                        mybir.AluOpType.mult)
                    nc.vector.tensor_copy(m[:, :, ZC:ZC + HEADS], num[:])

                    agg = psum_agg.tile([128, ZC + HEADS], F32, tag="agg")
                    for cc in range(C):
                        nc.tensor.matmul(
                            agg[:], of[:, cc, :], m[:, cc, :],
                            start=(cc == 0), stop=(cc == C - 1))

                    # fused D2 for this window's 128 nodes
                    msum = d2.tile([128, ZC + HEADS], F32, tag="msum")
                    nc.scalar.copy(msum[:], agg[:])
                    nm = d2.tile([128, HEADS], F32, tag="nm")
                    nc.vector.tensor_add(
                        nm[:], msum[:, ZC:ZC + HEADS], epsb[:])
                    rec = d2.tile([128, HEADS], F32, tag="rec")
                    nc.vector.reciprocal(rec[:], nm[:])
                    h1 = d2.tile([128, ZC], F32, tag="h1")
                    nc.vector.tensor_tensor(
                        h1[:].rearrange("p (h x) -> p h x", h=HEADS),
                        msum[:, 0:ZC].rearrange("p (h x) -> p h x", h=HEADS),
                        rec[:, :, None].to_broadcast((128, HEADS, HID)),
                        mybir.AluOpType.mult)
                    # elu(x)+1 = relu(x) + exp(-relu(-x)); -1 folded via crow
                    relu = d2.tile([128, ZC], F32, tag="relu")
                    nc.scalar.activation(relu[:], h1[:], AF.Relu)
                    rn = d2.tile([128, ZC], F32, tag="rn")
                    nc.scalar.activation(rn[:], h1[:], AF.Relu, scale=-1.0)
                    ex = d2.tile([128, ZC], F32, tag="ex")
                    nc.scalar.activation(ex[:], rn[:], AF.Exp, scale=-1.0)
                    h1e = d2.tile([128, ZC], F32, tag="h1e")
                    nc.vector.tensor_add(h1e[:], relu[:], ex[:])
                    h1eb = d2.tile([128, ZC], BF16, tag="h1eb")
                    nc.vector.tensor_copy(h1eb[:], h1e[:])

                    z2ps = psum_z2.tile([128, OUT + 2], F32, tag="z2p")
                    for kk in range(2):
                        tp = psum_tp.tile([128, 128], BF16, tag="tp")
                        nc.tensor.transpose(
                            tp[:], h1eb[:, kk * 128:(kk + 1) * 128], identb[:])
                        hT2 = d2.tile([128, 128], BF16, tag="hT2")
                        nc.scalar.copy(hT2[:], tp[:])
                        nc.tensor.matmul(
                            z2ps[:], hT2[:], wc2a[:] if kk == 0 else wc2b[:],
                            start=(kk == 0), stop=(kk == 1))

                    r0, r1 = wi * 128, (wi + 1) * 128
                    t2t = d2.tile([128, OUT + 2], BF16, tag="t2t")
                    nc.vector.tensor_tensor(
                        t2t[:, 0:OUT], z2ps[:, 0:OUT], crow[:, 0:OUT],
                        mybir.AluOpType.subtract)
                    nc.vector.tensor_copy(t2t[:, OUT:OUT + 1], onesb[:])
                    nc.vector.tensor_tensor(
                        t2t[:, OUT + 1:OUT + 2], z2ps[:, OUT:OUT + 1],
                        crow[:, OUT:OUT + 1], mybir.AluOpType.subtract)
                    nc.vector.tensor_tensor(
                        s2_all[:, wi:wi + 1], z2ps[:, OUT + 1:OUT + 2],
                        crow[:, OUT + 1:OUT + 2], mybir.AluOpType.subtract)
                    nc.sync.dma_start(t2_loc[r0:r1, :], t2t[:])

                for wi in range(NW):
                    stage.append(l1_stage_a(wi))
                    if len(stage) > PIPE:
                        l1_stage_b(stage.pop(0))
                while stage:
                    l1_stage_b(stage.pop(0))

            for (ps, sz) in pieces:
                nc.gpsimd.collective_compute(
                    "AllGather", mybir.AluOpType.bypass,
                    replica_groups=[list(range(NCORES))],
                    ins=[t2_loc[ps:ps + sz, :].opt()],
                    outs=[t2c_full[8 * ps:8 * ps + 8 * sz, :].opt()],
                )
                # expand compact rows into the 256B-stride gather table
                nc.sync.dma_start(
                    t2_full[8 * ps:8 * ps + 8 * sz, 0:T2C],
                    t2c_full[8 * ps:8 * ps + 8 * sz, :])

            # ------- L2 edge phase + output, software-pipelined ---------
            with (
                tc.tile_pool(name="l2", bufs=PIPE + 1) as l2,
                tc.tile_pool(name="l2g", bufs=PIPE + 2) as l2g,
                tc.tile_pool(name="l2o", bufs=PIPE + 2) as l2o,
                tc.tile_pool(name="l2m", bufs=2) as l2m,
                tc.tile_pool(name="psum_a2", bufs=2, space="PSUM") as psum_a2,
                tc.tile_pool(name="psum_q2", bufs=PIPE + 1,
                             space="PSUM") as psum_q2,
            ):
                qc = 0
                stage = []

                def l2_stage_a(wi):
                    nonlocal qc
                    base, KLw, KHw = windows[wi]
                    C = KLw + KHw
                    g = l2g.tile([128, C, 128], BF16, tag="g2")
                    for c0, c1, tab in ((0, KLw, 0), (KLw, C, HALF)):
                        if c1 == c0:
                            continue
                        b0, b1 = base + c0 * 128, base + c1 * 128
                        nc.gpsimd.dma_gather(
                            g[:, c0:c1, :], t2_full[tab:, :],
                            eidx_t[:, b0 // 16:b1 // 16],
                            num_idxs=(c1 - c0) * 128,
                            num_idxs_reg=(c1 - c0) * 128,
                            elem_size=128, single_packet=False,
                            queue_num=qc % 4); qc += 1
                    ot = l2o.tile([128, C, 128], OT_DT, tag="ot2")
                    nc.scalar.dma_start(ot[:], otab_in[:, base:base + C * 128])
                    of = l2o.tile([128, C, 128], OT_DT, tag="of2")
                    nc.scalar.dma_start(of[:], ofab_in[:, base:base + C * 128])
                    qd = psum_q2.tile([128, C, 1], F32, tag="qd2")
                    for cc in range(C):
                        nc.tensor.matmul(
                            qd[:, cc, :], ot[:, cc, :], s2_all[:, wi:wi + 1],
                            start=True, stop=True)
                    return (wi, C, g, of, qd)

                def l2_stage_b(st):
                    wi, C, g, of, qd = st
                    q = l2.tile([128, C, 1], F32, tag="q_2")
                    nc.vector.tensor_add(
                        q[:], g[:, :, OUT + 1:OUT + 2], qd[:])
                    ea = l2.tile([128, C, 1], BF16, tag="ea2")
                    nc.scalar.activation(ea[:], q[:], AF.Exp)
                    eb = l2.tile([128, C, 1], BF16, tag="eb2")
                    nc.scalar.activation(eb[:], q[:], AF.Exp, scale=0.01)
                    num = l2.tile([128, C, 1], BF16, tag="num2")
                    nc.vector.tensor_max(num[:], ea[:], eb[:])

                    m = l2m.tile([128, C, OUT + 1], BF16, tag="m2")
                    nc.vector.tensor_tensor(
                        m[:], g[:, :, 0:OUT + 1],
                        num[:].to_broadcast((128, C, OUT + 1)),
                        mybir.AluOpType.mult)

                    agg = psum_a2.tile([128, OUT + 1], F32, tag="agg2")
                    for cc in range(C):
                        nc.tensor.matmul(
                            agg[:], of[:, cc, :], m[:, cc, :],
                            start=(cc == 0), stop=(cc == C - 1))

                    msum = l2.tile([128, OUT + 1], F32, tag="bsum")
                    nc.scalar.copy(msum[:], agg[:])
                    nm = l2.tile([128, 1], F32, tag="bnm")
                    nc.vector.tensor_add(
                        nm[:], msum[:, OUT:OUT + 1], epsb[:, 0:1])
                    rec = l2.tile([128, 1], F32, tag="brec")
                    nc.vector.reciprocal(rec[:], nm[:])
                    otile = l2.tile([128, OUT], F32, tag="ot_out")
                    nc.vector.tensor_tensor(
                        otile[:].rearrange("p (a x) -> p a x", a=1),
                        msum[:, 0:OUT].rearrange("p (a x) -> p a x", a=1),
                        rec[:, :, None].to_broadcast((128, 1, OUT)),
                        mybir.AluOpType.mult)
                    nc.sync.dma_start(
                        out_ext[wi * 128:(wi + 1) * 128, :], otile[:])

                for wi in range(NW):
                    stage.append(l2_stage_a(wi))
                    if len(stage) > PIPE:
                        l2_stage_b(stage.pop(0))
                while stage:
                    l2_stage_b(stage.pop(0))

    nc.compile()
    return nc


# ----------------------------------------------------------------- frontend

_CACHE = {}


def _run(h, src, dst, W1, a1, W2, a2, trace=False):
    struct, in_maps, node_at = preprocess(h, src, dst, W1, a1, W2, a2)
    key = (struct["N"], struct["E"], struct["total_pos"],
           tuple(struct["windows"]))
    if key not in _CACHE:
        _CACHE[key] = build(struct)
    nc = _CACHE[key]
    res = run_bass_kernel_spmd(nc, in_maps, core_ids=list(range(NCORES)),
                               trace=trace)
    npc = struct["npc"]
    N, OUT = struct["N"], struct["OUT"]
    out = np.zeros((N, OUT), dtype=np.float32)
    for c in range(NCORES):
        oc = res.results[c]["out"]       # [rows, OUT] in (window, slot) order
        na = node_at[c]
        valid = na >= 0
        out[c * npc + na[valid]] = oc[np.flatnonzero(valid)]
    return out, res


def kernel(h, src, dst, W1, a1, W2, a2):
    h = np.asarray(h, dtype=np.float32)
    src = np.asarray(src, dtype=np.int32)
    dst = np.asarray(dst, dtype=np.int32)
    W1 = np.asarray(W1, dtype=np.float32)
    a1 = np.asarray(a1, dtype=np.float32)
    W2 = np.asarray(W2, dtype=np.float32)
    a2 = np.asarray(a2, dtype=np.float32)
    out, _ = _run(h, src, dst, W1, a1, W2, a2, trace=False)
    return out
